# revision 1
# baseline (speedup 1.0000x reference)
"""CompoundHeadAttention TRN2 kernel.

Full-input contract: kernel(**inputs) takes the unsharded tensors from
setup_inputs() and returns the full [1, 2048, 2048] float32 output.

Sharding (8 cores, tensor-parallel over the HK=8 kv heads):
  core h owns kv head h: its Wq/Wk/Wv column slice, its WG[h]/bG[h], and
  Wfc row-slice [h*256:(h+1)*256, :].  Each core computes its head's
  attention + its partial FC output [2048, 2048]; the host sums the 8
  partials and adds bfc (the "all-reduce" of the row-sharded FC).

Device-side math per core (N=2048, E=2048, D=64, G=4):
  QT  [128, n] = dup([Wq_h|Wq_h]^T q^T) + bq        (fp16 matmul, fp32 psum)
  KT  [128, s] = dup                                 (dup rows = row-tiling feed)
  V   [s, 64]  via VT matmul + PE transpose, ones column appended (M=65)
  QgT [128, n] per g-pair via WG row-tiled matmuls
  ST  [s=128, n=512] = KT_chunk^T QgT  (two row-tiled K=64 matmuls)
  PT  = exp(8*ST)  (ACT, scale folds the D**-0.5 softmax scale)
  causal mask: gpsimd affine_select zeroes PT where n < s (diagonal chunks)
  PV  [65, n] += Vones_chunk^T PT  (row 64 = softmax denominators)
  hidden = PV[0:64] * recip(PV[64])  (DVE + gpsimd partition_broadcast)
  out_partial[n, :] = hidden01^T Wfc[0:128] + hidden23^T Wfc[128:256]

Matmul dtypes: fp16 for the projections (inputs shipped as fp16),
float32r (1 cycle/row at N=512) for everything downstream.
"""

import os
import sys

import numpy as np

if "/opt/trn_rl_repo" not in sys.path and os.path.isdir("/opt/trn_rl_repo"):
    sys.path.insert(0, "/opt/trn_rl_repo")

import concourse.bass as bass  # noqa: E402
import concourse.mybir as mybir  # noqa: E402
import concourse.tile as tile  # noqa: E402
from concourse import bacc  # noqa: E402
from concourse import bass_utils  # noqa: E402

F32 = mybir.dt.float32
F32R = mybir.dt.float32r
F16 = mybir.dt.float16
AF = mybir.ActivationFunctionType

N = 2048
E = 2048
HK = 8
D = 64
G = 4
NB = 4        # 512-wide n-windows
SC_PER_NB = 4  # 128-wide s-chunks per window
NEG = -1e30


def build_program():
    nc = bacc.Bacc("TRN2", target_bir_lowering=False, debug=False,
                   enable_asserts=False)

    # ---- DRAM I/O ----
    qT = nc.dram_tensor("qT", [E, N], F16, kind="ExternalInput").ap()
    kT = nc.dram_tensor("kT", [E, N], F16, kind="ExternalInput").ap()
    vT = nc.dram_tensor("vT", [E, N], F16, kind="ExternalInput").ap()
    # weight chunk layout: [128, 16*M] — e-chunk ec occupies cols [M*ec, M*ec+M)
    wq = nc.dram_tensor("wq", [128, 16 * 128], F16, kind="ExternalInput").ap()
    wk = nc.dram_tensor("wk", [128, 16 * 128], F16, kind="ExternalInput").ap()
    wv = nc.dram_tensor("wv", [128, 16 * 64], F16, kind="ExternalInput").ap()
    bq2 = nc.dram_tensor("bq2", [128, 1], F32, kind="ExternalInput").ap()
    bk2 = nc.dram_tensor("bk2", [128, 1], F32, kind="ExternalInput").ap()
    bvv = nc.dram_tensor("bvv", [64, 1], F32, kind="ExternalInput").ap()
    wg = nc.dram_tensor("wg", [128, 256], F32R, kind="ExternalInput").ap()
    bg01 = nc.dram_tensor("bg01", [128, 1], F32, kind="ExternalInput").ap()
    bg23 = nc.dram_tensor("bg23", [128, 1], F32, kind="ExternalInput").ap()
    wfc = nc.dram_tensor("wfc", [256, E], F32R, kind="ExternalInput").ap()
    ident = nc.dram_tensor("ident", [128, 128], F32, kind="ExternalInput").ap()
    out = nc.dram_tensor("out", [N, E], F32, kind="ExternalOutput").ap()

    with tile.TileContext(nc) as tc:
        build_tile_kernel(tc, qT=qT, kT=kT, vT=vT, wq=wq, wk=wk, wv=wv,
                          bq2=bq2, bk2=bk2, bvv=bvv, wg=wg, bg01=bg01,
                          bg23=bg23, wfc=wfc, ident=ident, out=out)
    nc.compile()
    return nc


def build_tile_kernel(tc, *, qT, kT, vT, wq, wk, wv, bq2, bk2, bvv, wg,
                      bg01, bg23, wfc, ident, out):
    nc = tc.nc

    import contextlib
    ctx = contextlib.ExitStack()
    ctx.__enter__()
    cp = ctx.enter_context(tc.tile_pool(name="persist", bufs=1))

    def ptile(shape, dtype, name):
        return cp.tile(shape, dtype, tag=name, name=name)

    # ---- persistent constants in SBUF ----
    wq_sb = ptile([128, 16 * 128], F16, "wq_sb")
    wk_sb = ptile([128, 16 * 128], F16, "wk_sb")
    wv_sb = ptile([128, 16 * 64], F16, "wv_sb")
    wg_sb = ptile([128, 256], F32R, "wg_sb")
    wfc0_sb = ptile([128, E], F32R, "wfc0_sb")
    wfc1_sb = ptile([128, E], F32R, "wfc1_sb")
    id_sb = ptile([128, 128], F32, "id_sb")
    bq_sb = ptile([128, 1], F32, "bq_sb")
    bk_sb = ptile([128, 1], F32, "bk_sb")
    bv_sb = ptile([64, 1], F32, "bv_sb")
    bg01_sb = ptile([128, 1], F32, "bg01_sb")
    bg23_sb = ptile([128, 1], F32, "bg23_sb")
    ones_sb = ptile([128, 1], F32, "ones_sb")
    nc.vector.memset(ones_sb[:], 1.0)

    # only wq is needed before the first q rows can be consumed; the
    # remaining consts are interleaved into the Q loop (emit_consts).
    nc.sync.dma_start(wq_sb[:], wq[:])

    # per-window persistent activations
    qt_w = [ptile([128, 512], F32R, f"qt{j}") for j in range(NB)]
    kt_w = [ptile([128, 512], F32R, f"kt{j}") for j in range(NB)]
    vo_w = [ptile([128, 4 * 65], F32R, f"vo{j}") for j in range(NB)]
    qg01_w = [ptile([128, 512], F32R, f"qg01_{j}") for j in range(NB)]
    qg23_w = [ptile([128, 512], F32R, f"qg23_{j}") for j in range(NB)]
    hid01_w = [ptile([128, 512], F32R, f"hid01_{j}") for j in range(NB)]
    hid23_w = [ptile([128, 512], F32R, f"hid23_{j}") for j in range(NB)]

    with ctx:
        in_pool = ctx.enter_context(tc.tile_pool(name="in_pool", bufs=9))
        vt_pool = ctx.enter_context(tc.tile_pool(name="vt_pool", bufs=2))
        pt_pool = ctx.enter_context(tc.tile_pool(name="pt_pool", bufs=4))
        rec_pool = ctx.enter_context(tc.tile_pool(name="rec_pool", bufs=2))
        fco_pool = ctx.enter_context(tc.tile_pool(name="fco_pool", bufs=2))
        misc_ps = ctx.enter_context(
            tc.tile_pool(name="misc_ps", bufs=2, space="PSUM"))
        st_ps = ctx.enter_context(
            tc.tile_pool(name="st_ps", bufs=2, space="PSUM"))
        pv_ps = ctx.enter_context(
            tc.tile_pool(name="pv_ps", bufs=2, space="PSUM"))

        def emit_proj(P):
            """projections + G for window pair P ({0,1} or {2,3})"""
            pcol = bass.ds(P * 1024, 1024)
            wins = (2 * P, 2 * P + 1)

            # Q projection (both windows), dup'd output partitions
            q0_ps = misc_ps.tile([128, 512], F32, tag="mm", name="q0_ps")
            q1_ps = misc_ps.tile([128, 512], F32, tag="mm", name="q1_ps")
            for ec in range(16):
                q_in = in_pool.tile([128, 1024], F16, tag="qin", name="q_in")
                nc.sync.dma_start(q_in[:], qT[bass.ts(ec, 128), pcol])
                w = wq_sb[:, bass.ts(ec, 128)]
                nc.tensor.matmul(q0_ps[:], w, q_in[:, 0:512],
                                 start=(ec == 0), stop=(ec == 15))
                nc.tensor.matmul(q1_ps[:], w, q_in[:, 512:1024],
                                 start=(ec == 0), stop=(ec == 15))
                yield
            nc.scalar.activation(qt_w[wins[0]][:], q0_ps[:], AF.Identity, bias=bq_sb[:])
            nc.scalar.activation(qt_w[wins[1]][:], q1_ps[:], AF.Identity, bias=bq_sb[:])

            # K projection (both windows)
            k0_ps = misc_ps.tile([128, 512], F32, tag="mm", name="k0_ps")
            k1_ps = misc_ps.tile([128, 512], F32, tag="mm", name="k1_ps")
            for ec in range(16):
                k_in = in_pool.tile([128, 1024], F16, tag="kin", name="k_in")
                nc.sync.dma_start(k_in[:], kT[bass.ts(ec, 128), pcol])
                w = wk_sb[:, bass.ts(ec, 128)]
                nc.tensor.matmul(k0_ps[:], w, k_in[:, 0:512],
                                 start=(ec == 0), stop=(ec == 15))
                nc.tensor.matmul(k1_ps[:], w, k_in[:, 512:1024],
                                 start=(ec == 0), stop=(ec == 15))
                yield
            nc.scalar.activation(kt_w[wins[0]][:], k0_ps[:], AF.Identity, bias=bk_sb[:])
            nc.scalar.activation(kt_w[wins[1]][:], k1_ps[:], AF.Identity, bias=bk_sb[:])

            # V projection: VT then PE-transpose to V (+ ones column)
            v0_ps = misc_ps.tile([64, 512], F32, tag="mm", name="v0_ps")
            v1_ps = misc_ps.tile([64, 512], F32, tag="mm", name="v1_ps")
            for ec in range(16):
                v_in = in_pool.tile([128, 1024], F16, tag="vin", name="v_in")
                nc.sync.dma_start(v_in[:], vT[bass.ts(ec, 128), pcol])
                w = wv_sb[:, bass.ts(ec, 64)]
                nc.tensor.matmul(v0_ps[:], w, v_in[:, 0:512],
                                 start=(ec == 0), stop=(ec == 15))
                nc.tensor.matmul(v1_ps[:], w, v_in[:, 512:1024],
                                 start=(ec == 0), stop=(ec == 15))
                yield
            for wi, v_ps in ((wins[0], v0_ps), (wins[1], v1_ps)):
                vt_sb = vt_pool.tile([64, 512], F32, tag="vt", name="vt_sb")
                nc.scalar.activation(vt_sb[:], v_ps[:], AF.Identity, bias=bv_sb[:])
                tr_ps = misc_ps.tile([128, 256], F32, tag="mm", name="tr_ps")
                for t in range(4):
                    nc.tensor.transpose(tr_ps[:, bass.ts(t, 64)],
                                        vt_sb[:, bass.ts(t, 128)],
                                        id_sb[0:64, 0:64])
                for t in range(4):
                    nc.vector.tensor_copy(vo_w[wi][:, t * 65:t * 65 + 64],
                                          tr_ps[:, bass.ts(t, 64)])
                    nc.vector.tensor_copy(
                        vo_w[wi][:, t * 65 + 64:t * 65 + 65], ones_sb[:])

            # G transform per window, row-tiled pair01 / pair23
            for wi in wins:
                g01_ps = misc_ps.tile([128, 512], F32, tag="mm", name="g01_ps")
                g23_ps = misc_ps.tile([128, 512], F32, tag="mm", name="g23_ps")
                nc.tensor.matmul(g01_ps[:], wg_sb[0:64, 0:128],
                                 qt_w[wi][0:64, :], start=True, stop=True)
                nc.tensor.matmul(g23_ps[:], wg_sb[64:128, 128:256],
                                 qt_w[wi][64:128, :], start=True, stop=True)
                nc.scalar.activation(qg01_w[wi][:], g01_ps[:], AF.Identity,
                                     bias=bg01_sb[:])
                nc.scalar.activation(qg23_w[wi][:], g23_ps[:], AF.Identity,
                                     bias=bg23_sb[:])
                yield

        def emit_attn(j):
            klast = 4 * j + 3
            for (qg, hid) in ((qg01_w[j], hid01_w[j]),
                              (qg23_w[j], hid23_w[j])):
                pv_a = pv_ps.tile([65, 512], F32, tag="pv", name="pv_a")
                pv_b = pv_ps.tile([65, 512], F32, tag="pv", name="pv_b")
                for k in range(klast + 1):
                    kt_c = kt_w[k // 4]
                    ks = bass.ts(k % 4, 128)
                    # causal trim: diagonal chunk k covers n-cols [off, 512).
                    # i=3 is padded to 256 wide: float32r matmuls below 256
                    # moving cols run at 1/4 rate, so N=128 costs as much as
                    # N=512 while N=256 costs half.
                    i = k - 4 * j
                    off = max(0, 128 * i)
                    if off == 384:
                        off = 256
                    st = st_ps.tile([128, 1024], F32, tag="st", name="st")
                    nc.tensor.matmul(st[:, off:512], kt_c[0:64, ks],
                                     qg[0:64, off:512], start=True, stop=True)
                    nc.tensor.matmul(st[:, 512 + off:1024], kt_c[64:128, ks],
                                     qg[64:128, off:512],
                                     start=True, stop=True)
                    pt = pt_pool.tile([128, 1024], F32R, tag="pt", name="pt")
                    st3 = st[:].rearrange("p (h c) -> p h c", c=512)
                    pt3 = pt[:].rearrange("p (h c) -> p h c", c=512)
                    nc.scalar.activation(pt3[:, :, off:512],
                                         st3[:, :, off:512],
                                         AF.Exp, scale=8.0)
                    if i >= 0:
                        # mask region [off, 128*i+128): keep where global
                        # col >= s + 128*i, i.e. local c' >= s + (128*i-off)
                        mw = 128 * i + 128 - off
                        nc.gpsimd.affine_select(
                            out=pt3[:, :, off:off + mw],
                            in_=pt3[:, :, off:off + mw],
                            compare_op=mybir.AluOpType.is_ge,
                            fill=0.0, base=-(128 * i - off),
                            pattern=[[0, 2], [1, mw]],
                            channel_multiplier=-1)
                    vo_c = vo_w[k // 4]
                    vsl = vo_c[:, (k % 4) * 65:(k % 4) * 65 + 65]
                    nc.tensor.matmul(pv_a[:, off:512], vsl, pt[:, off:512],
                                     start=(k == 0), stop=(k == klast))
                    nc.tensor.matmul(pv_b[:, off:512], vsl,
                                     pt[:, 512 + off:1024],
                                     start=(k == 0), stop=(k == klast))
                    yield
                # normalize: hidden[g-half] = pv[0:64] * 1/pv[64]
                for half, pv in ((0, pv_a), (1, pv_b)):
                    rec = rec_pool.tile([1, 512], F32, tag="rec", name="rec")
                    nc.vector.reciprocal(rec[:], pv[64:65, :])
                    recr = rec_pool.tile([64, 512], F32, tag="recr",
                                         name="recr")
                    nc.gpsimd.partition_broadcast(recr[:], rec[:])
                    nc.vector.tensor_mul(hid[half * 64:half * 64 + 64, :],
                                         pv[0:64, :], recr[:])

        def emit_fc(j):
            for m in range(4):
                msl = bass.ts(m, 128)
                stage = fco_pool.tile([128, 2048], F32, tag="fco",
                                      name="stage")
                for eo in range(4):
                    fc_ps = misc_ps.tile([128, 512], F32, tag="mm",
                                         name="fc_ps")
                    nc.tensor.matmul(fc_ps[:], hid01_w[j][:, msl],
                                     wfc0_sb[:, bass.ts(eo, 512)],
                                     start=True, stop=False)
                    nc.tensor.matmul(fc_ps[:], hid23_w[j][:, msl],
                                     wfc1_sb[:, bass.ts(eo, 512)],
                                     start=False, stop=True)
                    nc.vector.tensor_copy(stage[:, bass.ts(eo, 512)],
                                          fc_ps[:])
                    yield
                nc.sync.dma_start(
                    out[512 * j + 128 * m: 512 * j + 128 * m + 128, :],
                    stage[:])

        def emit_consts():
            for dst, srcap in ((wk_sb, wk), (wv_sb, wv), (bq_sb, bq2),
                               (bk_sb, bk2), (bv_sb, bvv), (wg_sb, wg),
                               (id_sb, ident), (bg01_sb, bg01),
                               (bg23_sb, bg23)):
                nc.sync.dma_start(dst[:], srcap[:])
                yield

        from itertools import chain as ichain

        def drain(g):
            for _ in g:
                pass

        def rr(pairs):
            """round-robin emission: [(generator, steps_per_turn)]"""
            live = [[g, w] for g, w in pairs]
            while live:
                for gw in list(live):
                    g, w = gw
                    try:
                        for _ in range(w):
                            next(g)
                    except StopIteration:
                        live.remove(gw)

        def emit_wfc():
            nc.sync.dma_start(wfc0_sb[:], wfc[0:128, :])
            nc.sync.dma_start(wfc1_sb[:], wfc[128:256, :])
            yield

        # Phase A: pair-0 projections (DMA-bound ramp); remaining consts
        # trickle in between the first q-row loads
        rr([(emit_proj(0), 1), (emit_consts(), 1)])
        # Phase B: window-0 attention interleaved with pair-1 projections
        # (DMA hides under ACT-bound attention)
        rr([(emit_attn(0), 1), (ichain(emit_proj(1), emit_wfc()), 2)])
        # Later windows: attention with FC of completed windows as PE filler
        rr([(emit_attn(1), 1), (emit_fc(0), 1)])
        rr([(emit_attn(2), 1), (emit_fc(1), 1)])
        rr([(emit_attn(3), 1), (emit_fc(2), 1)])
        drain(emit_fc(3))


def shard_inputs(inputs):
    """full inputs -> list of 8 per-core in_maps (numpy, device layouts)"""
    f16 = np.float16
    f32 = np.float32
    q = np.asarray(inputs["q"], f32)[0]
    k = np.asarray(inputs["k"], f32)[0]
    v = np.asarray(inputs["v"], f32)[0]
    Wq = np.asarray(inputs["Wq"], f32)
    Wk = np.asarray(inputs["Wk"], f32)
    Wv = np.asarray(inputs["Wv"], f32)
    bq = np.asarray(inputs["bq"], f32)
    bk = np.asarray(inputs["bk"], f32)
    bv = np.asarray(inputs["bv"], f32)
    WG = np.asarray(inputs["WG"], f32)
    bG = np.asarray(inputs["bG"], f32)
    Wfc = np.asarray(inputs["Wfc"], f32)

    qT = np.ascontiguousarray(q.T.astype(f16))
    kT = np.ascontiguousarray(k.T.astype(f16))
    vT = np.ascontiguousarray(v.T.astype(f16))
    ident = np.eye(128, dtype=f32)

    def chunked(w):
        # [E, M] -> [128, 16*M]: e-chunk ec at cols [M*ec, M*ec+M)
        M = w.shape[1]
        return np.ascontiguousarray(
            w.reshape(16, 128, M).transpose(1, 0, 2).reshape(128, 16 * M))

    maps = []
    for h in range(HK):
        sl = slice(h * D, (h + 1) * D)
        wq_h = Wq[:, sl]
        wk_h = Wk[:, sl]
        wv_h = Wv[:, sl]
        m = {
            "qT": qT, "kT": kT, "vT": vT,
            "wq": chunked(np.concatenate([wq_h, wq_h], 1)).astype(f16),
            "wk": chunked(np.concatenate([wk_h, wk_h], 1)).astype(f16),
            "wv": chunked(wv_h).astype(f16),
            "bq2": np.concatenate([bq[sl], bq[sl]]).reshape(128, 1).copy(),
            "bk2": np.concatenate([bk[sl], bk[sl]]).reshape(128, 1).copy(),
            "bvv": bv[sl].reshape(64, 1).copy(),
            "wg": np.concatenate([WG[h], WG[h]], 0).copy(),  # [128, 256]
            "bg01": bG[h, 0:128].reshape(128, 1).copy(),
            "bg23": bG[h, 128:256].reshape(128, 1).copy(),
            "wfc": Wfc[h * 256:(h + 1) * 256, :].copy(),
            "ident": ident,
        }
        maps.append(m)
    return maps


_compiled = None
last_results = None


def get_compiled():
    global _compiled
    if _compiled is None:
        _compiled = build_program()
    return _compiled


def kernel(**inputs):
    global last_results
    nc = get_compiled()
    in_maps = shard_inputs(inputs)
    last_results = bass_utils.run_bass_kernel_spmd(
        nc, in_maps, core_ids=list(range(8)))
    bfc = np.asarray(inputs["bfc"], np.float32)
    acc = np.zeros((N, E), np.float64)
    for res in last_results.results:
        acc += res["out"].astype(np.float64)
    full = (acc + bfc[None, :].astype(np.float64)).astype(np.float32)
    return full.reshape(1, N, E)



# revision 22
# speedup vs baseline: 1.0432x; 1.0432x over previous
"""CompoundHeadAttention TRN2 kernel (v2).

Full-input contract: kernel(**inputs) takes the unsharded tensors from
setup_inputs() and returns the full [1, 2048, 2048] float32 output.

Sharding (8 cores, tensor-parallel over the HK=8 kv heads):
  core h owns kv head h: its Wq/Wk/Wv column slice, its WG[h]/bG[h], and
  Wfc row-slice [h*256:(h+1)*256, :].  Each core computes its head's
  attention + its partial FC output [2048, 2048] in fp16; the host sums
  the 8 partials and adds bfc (the "all-reduce" of the row-sharded FC).

v2 device-side design (N=2048, E=2048, D=64, G=4 per core):
  - inputs qT/kT/vT [E, N] fp16 loaded as [128, 4, 1024] "quads", 4 per
    (tensor, window-pair); triggers spread over sync/gpsimd/vector queues
  - projections fp16, M=64 (no partition dup): psum [64, 512] per window
  - G transform fp16 from qt; qg0..3 stored as separate [64, 512] tiles
    at base partition 0 so ST needs no duplicated kt rows
  - ST fp16 [128 s, n] with exact 128-col causal trim (fp16 has no
    small-N matmul penalty); exp on ACT -> pt bf16; diagonal mask via
    DVE multiply with a precomputed triangular bf16 tile
  - PV bf16 accumulated [65, n] (row 64 = softmax denominators via a
    ones column in vo); PV lags ST by one chunk to hide ACT latency
  - normalize: reciprocal_approx_fast (DVE) + partition_broadcast (Pool)
    + tensor_mul -> hid fp16
  - FC fp16: hid [128, 128] stationary x wfc [128, 512]; psum->stage
    fp16 copies alternate DVE/Pool; out DMA per 128-row chunk
"""

import os
import sys

import numpy as np

if "/opt/trn_rl_repo" not in sys.path and os.path.isdir("/opt/trn_rl_repo"):
    sys.path.insert(0, "/opt/trn_rl_repo")

import concourse.bass as bass  # noqa: E402
import concourse.mybir as mybir  # noqa: E402
import concourse.tile as tile  # noqa: E402
from concourse import bacc  # noqa: E402
from concourse import bass_utils  # noqa: E402

F32 = mybir.dt.float32
F16 = mybir.dt.float16
BF16 = mybir.dt.bfloat16
AF = mybir.ActivationFunctionType

N = 2048
E = 2048
HK = 8
D = 64
G = 4
NB = 4         # 512-wide n-windows


def build_program():
    nc = bacc.Bacc("TRN2", target_bir_lowering=False, debug=False,
                   enable_asserts=False)

    # ---- DRAM I/O ----
    qT = nc.dram_tensor("qT", [E, N], F16, kind="ExternalInput").ap()
    kT = nc.dram_tensor("kT", [E, N], F16, kind="ExternalInput").ap()
    vT = nc.dram_tensor("vT", [E, N], F16, kind="ExternalInput").ap()
    # weight chunk layout: [128, 16*64] — e-chunk ec occupies cols [64ec, 64ec+64)
    wq = nc.dram_tensor("wq", [128, 16 * 64], F16, kind="ExternalInput").ap()
    wk = nc.dram_tensor("wk", [128, 16 * 64], F16, kind="ExternalInput").ap()
    wv = nc.dram_tensor("wv", [128, 16 * 64], F16, kind="ExternalInput").ap()
    bq = nc.dram_tensor("bq", [64, 1], F32, kind="ExternalInput").ap()
    bk = nc.dram_tensor("bk", [64, 1], F32, kind="ExternalInput").ap()
    bv = nc.dram_tensor("bv", [64, 1], F32, kind="ExternalInput").ap()
    wg = nc.dram_tensor("wg", [64, 256], F16, kind="ExternalInput").ap()
    bg = nc.dram_tensor("bg", [64, 4], F32, kind="ExternalInput").ap()
    wfc = nc.dram_tensor("wfc", [256, E], F16, kind="ExternalInput").ap()
    out = nc.dram_tensor("out", [N, E], F16, kind="ExternalOutput").ap()

    dumps = None
    if os.environ.get("KDUMP"):
        dumps = {
            "d_kt0": nc.dram_tensor("d_kt0", [64, 512], F16,
                                    kind="ExternalOutput").ap(),
            "d_qg00": nc.dram_tensor("d_qg00", [64, 512], F16,
                                     kind="ExternalOutput").ap(),
            "d_vo0": nc.dram_tensor("d_vo0", [128, 4, 65], BF16,
                                    kind="ExternalOutput").ap(),
            "d_hid01_0": nc.dram_tensor("d_hid01_0", [128, 512], F16,
                                        kind="ExternalOutput").ap(),
            "d_rec": nc.dram_tensor("d_rec", [64, 512], F32,
                                    kind="ExternalOutput").ap(),
        }

    with tile.TileContext(nc) as tc:
        build_tile_kernel(tc, qT=qT, kT=kT, vT=vT, wq=wq, wk=wk, wv=wv,
                          bq=bq, bk=bk, bv=bv, wg=wg, bg=bg, wfc=wfc,
                          out=out, dumps=dumps)
    nc.compile()
    return nc


def build_tile_kernel(tc, *, qT, kT, vT, wq, wk, wv, bq, bk, bv, wg, bg,
                      wfc, out, dumps=None):
    nc = tc.nc

    import contextlib
    ctx = contextlib.ExitStack()
    ctx.__enter__()
    cp = ctx.enter_context(tc.tile_pool(name="persist", bufs=1))

    def ptile(shape, dtype, name):
        return cp.tile(shape, dtype, tag=name, name=name)

    # ---- persistent constants in SBUF ----
    wq_sb = ptile([128, 16 * 64], F16, "wq_sb")
    wk_sb = ptile([128, 16 * 64], F16, "wk_sb")
    wv_sb = ptile([128, 16 * 64], F16, "wv_sb")
    wg_sb = ptile([64, 256], F16, "wg_sb")
    wfc0_sb = ptile([128, E], F16, "wfc0_sb")
    wfc1_sb = ptile([128, E], F16, "wfc1_sb")
    bq_sb = ptile([64, 1], F32, "bq_sb")
    bk_sb = ptile([64, 1], F32, "bk_sb")
    bv_sb = ptile([64, 1], F32, "bv_sb")
    bg_sb = ptile([64, 4], F32, "bg_sb")
    # causal mask constant: mask[s, n_local] = 1 if n_local >= s else 0,
    # duplicated side by side for the two g-halves of a pair
    mask_sb = ptile([128, 256], BF16, "mask_sb")
    nc.vector.memset(mask_sb[:], 1.0)
    # exp logit shift (cancels in softmax; keeps exp within fp16 range)
    eshift_sb = ptile([128, 1], F32, "eshift_sb")
    nc.vector.memset(eshift_sb[:], -35.0)
    mask3 = mask_sb[:].rearrange("p (h c) -> p h c", c=128)
    nc.gpsimd.affine_select(
        out=mask3, in_=mask3, compare_op=mybir.AluOpType.is_ge,
        fill=0.0, base=0, pattern=[[0, 2], [1, 128]], channel_multiplier=-1)

    # per-window persistent activations
    kt_w = [ptile([64, 512], F16, f"kt{j}") for j in range(NB)]
    # one tile per 128-token V chunk: DMA transpose writes at offset 0,
    # col 64 holds the ones column (softmax denominator row of PV)
    vo_w = [[ptile([128, 65], BF16, f"vo{j}_{c}") for c in range(4)]
            for j in range(NB)]
    for j in range(NB):
        for c in range(4):
            nc.vector.memset(vo_w[j][c][:, 64:65], 1.0)
    qg_w = [[ptile([64, 512], F16, f"qg{j}_{g}") for g in range(G)]
            for j in range(NB)]
    hid01_w = [ptile([128, 512], F16, f"hid01_{j}") for j in range(NB)]
    hid23_w = [ptile([128, 512], F16, f"hid23_{j}") for j in range(NB)]

    with ctx:
        in_pool = ctx.enter_context(tc.tile_pool(name="in_pool", bufs=5))
        qt_pool = ctx.enter_context(tc.tile_pool(name="qt_pool", bufs=2))
        vt_pool = ctx.enter_context(tc.tile_pool(name="vt_pool", bufs=2))
        pt_pool = ctx.enter_context(tc.tile_pool(name="pt_pool", bufs=3))
        rec_pool = ctx.enter_context(tc.tile_pool(name="rec_pool", bufs=2))
        stage_pool = ctx.enter_context(tc.tile_pool(name="stage", bufs=2))
        ps = ctx.enter_context(
            tc.tile_pool(name="ps", bufs=2, space="PSUM"))

        # quad tiles, filled by emit_dma, consumed by emit_proj
        quads = {t: [None] * 8 for t in "qkv"}

        def load_quad(t, idx, eng, src, P, qd):
            tl = in_pool.tile([128, 4, 1024], F16, tag=f"{t}quad",
                              name=f"{t}in_{idx}")
            quads[t][idx] = tl
            sl = src[qd * 512:(qd + 1) * 512,
                     bass.ds(P * 1024, 1024)]
            eng.dma_start(tl[:], sl.rearrange("(e p) c -> p e c", p=128))

        def emit_dma_head():
            """wq + pair-0 input quads + early consts (phase A)."""
            nc.sync.dma_start(wq_sb[:], wq[:])
            for qd in range(4):
                load_quad("q", qd, nc.sync, qT, 0, qd)
                yield
            nc.sync.dma_start(wk_sb[:], wk[:])
            for qd in range(4):
                load_quad("k", qd, nc.gpsimd, kT, 0, qd)
                yield
            nc.sync.dma_start(wv_sb[:], wv[:])
            for qd in range(4):
                load_quad("v", qd, nc.scalar, vT, 0, qd)
                yield
            for dst, src in ((bq_sb, bq), (bk_sb, bk), (bv_sb, bv),
                             (wg_sb, wg), (bg_sb, bg)):
                nc.sync.dma_start(dst[:], src[:])
            yield

        def emit_dma_tail():
            """pair-1 input quads + wfc (phase B, overlapped)."""
            for qd in range(4):
                load_quad("q", 4 + qd, nc.sync, qT, 1, qd)
                yield
            for qd in range(4):
                load_quad("k", 4 + qd, nc.gpsimd, kT, 1, qd)
                yield
            for qd in range(4):
                load_quad("v", 4 + qd, nc.scalar, vT, 1, qd)
                yield
            nc.sync.dma_start(wfc0_sb[:], wfc[0:128, :])
            yield
            nc.sync.dma_start(wfc1_sb[:], wfc[128:256, :])
            yield

        dma_gens = {}

        def ensure_quad(t, idx):
            gen = dma_gens[0] if idx < 4 else dma_gens[1]
            while quads[t][idx] is None:
                try:
                    next(gen)
                except StopIteration:
                    break
            assert quads[t][idx] is not None

        def emit_proj(P):
            """projections + G + V transpose for window pair P."""
            wins = (2 * P, 2 * P + 1)

            # --- Q ---
            q0_ps = ps.tile([64, 512], F32, tag="mm", name="q0_ps")
            q1_ps = ps.tile([64, 512], F32, tag="mm", name="q1_ps")
            for ec in range(16):
                ensure_quad("q", P * 4 + ec // 4)
                quad = quads["q"][P * 4 + ec // 4]
                w = wq_sb[:, bass.ts(ec, 64)]
                nc.tensor.matmul(q0_ps[:], w, quad[:, ec % 4, 0:512],
                                 start=(ec == 0), stop=(ec == 15))
                yield
                nc.tensor.matmul(q1_ps[:], w, quad[:, ec % 4, 512:1024],
                                 start=(ec == 0), stop=(ec == 15))
                yield
            qt0 = qt_pool.tile([64, 512], F16, tag="qt", name="qt0")
            qt1 = qt_pool.tile([64, 512], F16, tag="qt", name="qt1")
            nc.scalar.activation(qt0[:], q0_ps[:], AF.Identity, bias=bq_sb[:])
            nc.scalar.activation(qt1[:], q1_ps[:], AF.Identity, bias=bq_sb[:])
            # --- G transform ---
            for wi, qt in ((wins[0], qt0), (wins[1], qt1)):
                g01_ps = ps.tile([128, 512], F32, tag="mm", name="g01_ps")
                nc.tensor.matmul(g01_ps[:], wg_sb[:, 0:128], qt[:],
                                 start=True, stop=True)
                yield
                g23_ps = ps.tile([128, 512], F32, tag="mm", name="g23_ps")
                nc.tensor.matmul(g23_ps[:], wg_sb[:, 128:256], qt[:],
                                 start=True, stop=True)
                yield
                for g in range(4):
                    src = (g01_ps if g < 2 else g23_ps)
                    nc.scalar.activation(
                        qg_w[wi][g][:], src[(g % 2) * 64:(g % 2) * 64 + 64, :],
                        AF.Identity, bias=bg_sb[:, g:g + 1])

            # --- K ---
            k0_ps = ps.tile([64, 512], F32, tag="mm", name="k0_ps")
            k1_ps = ps.tile([64, 512], F32, tag="mm", name="k1_ps")
            for ec in range(16):
                ensure_quad("k", P * 4 + ec // 4)
                quad = quads["k"][P * 4 + ec // 4]
                w = wk_sb[:, bass.ts(ec, 64)]
                nc.tensor.matmul(k0_ps[:], w, quad[:, ec % 4, 0:512],
                                 start=(ec == 0), stop=(ec == 15))
                yield
                nc.tensor.matmul(k1_ps[:], w, quad[:, ec % 4, 512:1024],
                                 start=(ec == 0), stop=(ec == 15))
                yield
            nc.scalar.activation(kt_w[wins[0]][:], k0_ps[:], AF.Identity,
                                 bias=bk_sb[:])
            nc.scalar.activation(kt_w[wins[1]][:], k1_ps[:], AF.Identity,
                                 bias=bk_sb[:])

            # --- V ---
            v0_ps = ps.tile([64, 512], F32, tag="mm", name="v0_ps")
            v1_ps = ps.tile([64, 512], F32, tag="mm", name="v1_ps")
            for ec in range(16):
                ensure_quad("v", P * 4 + ec // 4)
                quad = quads["v"][P * 4 + ec // 4]
                w = wv_sb[:, bass.ts(ec, 64)]
                nc.tensor.matmul(v0_ps[:], w, quad[:, ec % 4, 0:512],
                                 start=(ec == 0), stop=(ec == 15))
                yield
                nc.tensor.matmul(v1_ps[:], w, quad[:, ec % 4, 512:1024],
                                 start=(ec == 0), stop=(ec == 15))
                yield
            for wi, v_ps in ((wins[0], v0_ps), (wins[1], v1_ps)):
                vt_sb = vt_pool.tile([64, 512], BF16, tag="vt", name="vt_sb")
                nc.scalar.activation(vt_sb[:], v_ps[:], AF.Identity,
                                     bias=bv_sb[:])
                for c in range(4):
                    nc.sync.dma_start_transpose(
                        vo_w[wi][c][:, 0:64],
                        vt_sb[:, bass.ts(c, 128)])
                yield

        def emit_attn(j):
            klast = 4 * j + 3
            for p in range(2):  # g-pairs (2p, 2p+1)
                pv_a = ps.tile([65, 512], F32, tag="pv", name="pv_a")
                pv_b = ps.tile([65, 512], F32, tag="pv", name="pv_b")
                pending = None

                def flush(pend):
                    pt, k, off = pend
                    vsl = vo_w[k // 4][k % 4][:, 0:65]
                    nc.tensor.matmul(pv_a[:, off:512], vsl, pt[:, off:512],
                                     start=(k == 0), stop=(k == klast))
                    nc.tensor.matmul(pv_b[:, off:512], vsl,
                                     pt[:, 512 + off:1024],
                                     start=(k == 0), stop=(k == klast))

                for k in range(klast + 1):
                    kc = kt_w[k // 4][:, bass.ts(k % 4, 128)]
                    i = k - 4 * j
                    off = max(0, 128 * i)
                    st = ps.tile([128, 1024], F32, tag="st", name="st")
                    nc.tensor.matmul(st[:, off:512], kc,
                                     qg_w[j][2 * p][:, off:512],
                                     start=True, stop=True)
                    yield
                    nc.tensor.matmul(st[:, 512 + off:1024], kc,
                                     qg_w[j][2 * p + 1][:, off:512],
                                     start=True, stop=True)
                    yield
                    pt = pt_pool.tile([128, 1024], BF16, tag="pt", name="pt")
                    st3 = st[:].rearrange("p (h c) -> p h c", c=512)
                    pt3 = pt[:].rearrange("p (h c) -> p h c", c=512)
                    # exp(8S - 35): the shift cancels in the softmax ratio
                    # and keeps all exp outputs within fp16/bf16 range (HW
                    # ACT saturates 16-bit outputs near the fp16 max)
                    nc.scalar.activation(pt3[:, :, off:512],
                                         st3[:, :, off:512],
                                         AF.Exp, scale=8.0,
                                         bias=eshift_sb[:])
                    if i >= 0:
                        # zero out below-diagonal cols [off, off+128)
                        nc.vector.tensor_mul(pt3[:, :, off:off + 128],
                                             pt3[:, :, off:off + 128],
                                             mask3)
                    if pending is not None:
                        flush(pending)
                        yield
                    pending = (pt, k, off)
                flush(pending)
                yield
                # normalize: hid[half] = pv[0:64] * 1/pv[64]
                hid = hid01_w[j] if p == 0 else hid23_w[j]
                for half, pv in ((0, pv_a), (1, pv_b)):
                    # custom-DVE recip can't read PSUM on HW: stage to SBUF
                    den = rec_pool.tile([1, 512], F32, tag="den", name="den")
                    nc.vector.tensor_copy(den[:], pv[64:65, :])
                    rec = rec_pool.tile([1, 512], F32, tag="rec", name="rec")
                    nc.vector.reciprocal_approx_fast(rec[:], den[:])
                    recr = rec_pool.tile([64, 512], F32, tag="recr",
                                         name="recr")
                    nc.gpsimd.partition_broadcast(recr[:], rec[:])
                    if dumps is not None and j == 0 and p == 0 and half == 0:
                        nc.sync.dma_start(dumps["d_rec"][:], recr[:])
                    nc.vector.tensor_mul(hid[half * 64:half * 64 + 64, :],
                                         pv[0:64, :], recr[:])

        def emit_fc(j):
            for m in range(4):
                msl = bass.ts(m, 128)
                stage = stage_pool.tile([128, 2048], F16, tag="fco",
                                        name="stage")
                for eo in range(4):
                    fc_ps = ps.tile([128, 512], F32, tag="mm",
                                    name="fc_ps")
                    nc.tensor.matmul(fc_ps[:], hid01_w[j][:, msl],
                                     wfc0_sb[:, bass.ts(eo, 512)],
                                     start=True, stop=False)
                    yield
                    nc.tensor.matmul(fc_ps[:], hid23_w[j][:, msl],
                                     wfc1_sb[:, bass.ts(eo, 512)],
                                     start=False, stop=True)
                    yield
                    if eo % 2 == 0:
                        nc.vector.tensor_copy(stage[:, bass.ts(eo, 512)],
                                              fc_ps[:])
                    else:
                        nc.scalar.copy(stage[:, bass.ts(eo, 512)], fc_ps[:])
                nc.sync.dma_start(
                    out[512 * j + 128 * m: 512 * j + 128 * m + 128, :],
                    stage[:])

        from itertools import chain as ichain

        def drain(g):
            for _ in g:
                pass

        def rr(pairs):
            """round-robin emission: [(generator, steps_per_turn)]"""
            live = [[g, w] for g, w in pairs]
            while live:
                for gw in list(live):
                    g, w = gw
                    try:
                        for _ in range(w):
                            next(g)
                    except StopIteration:
                        live.remove(gw)

        # Phase A: DMA ramp + pair-0 projections
        dma_gens[0] = emit_dma_head()
        dma_gens[1] = emit_dma_tail()
        rr([(dma_gens[0], 2), (emit_proj(0), 3)])
        # Phase B: window-0/1 attention + pair-1 projections + tail DMAs
        rr([(ichain(emit_attn(0), emit_attn(1)), 2),
            (emit_proj(1), 3),
            (dma_gens[1], 1)])
        # Later: attention with FC of completed windows as PE filler
        rr([(emit_attn(2), 5), (emit_fc(0), 1)])
        rr([(emit_attn(3), 5), (ichain(emit_fc(1), emit_fc(2)), 2)])
        drain(emit_fc(3))

        if dumps is not None:
            nc.sync.dma_start(dumps["d_kt0"][:], kt_w[0][0:64, :])
            nc.sync.dma_start(dumps["d_qg00"][:], qg_w[0][0][0:64, :])
            for c in range(4):
                nc.sync.dma_start(dumps["d_vo0"][:, c, :], vo_w[0][c][:, :])
            nc.sync.dma_start(dumps["d_hid01_0"][:], hid01_w[0][:, :])


def shard_inputs(inputs):
    """full inputs -> list of 8 per-core in_maps (numpy, device layouts)"""
    f16 = np.float16
    f32 = np.float32
    q = np.asarray(inputs["q"], f32)[0]
    k = np.asarray(inputs["k"], f32)[0]
    v = np.asarray(inputs["v"], f32)[0]
    Wq = np.asarray(inputs["Wq"], f32)
    Wk = np.asarray(inputs["Wk"], f32)
    Wv = np.asarray(inputs["Wv"], f32)
    bq = np.asarray(inputs["bq"], f32)
    bk = np.asarray(inputs["bk"], f32)
    bv = np.asarray(inputs["bv"], f32)
    WG = np.asarray(inputs["WG"], f32)
    bG = np.asarray(inputs["bG"], f32)
    Wfc = np.asarray(inputs["Wfc"], f32)

    qT = np.ascontiguousarray(q.T.astype(f16))
    kT = np.ascontiguousarray(k.T.astype(f16))
    vT = np.ascontiguousarray(v.T.astype(f16))

    def chunked(w):
        # [E, 64] -> [128, 16*64]: e-chunk ec at cols [64ec, 64ec+64)
        M = w.shape[1]
        return np.ascontiguousarray(
            w.reshape(16, 128, M).transpose(1, 0, 2).reshape(128, 16 * M))

    maps = []
    for h in range(HK):
        sl = slice(h * D, (h + 1) * D)
        m = {
            "qT": qT, "kT": kT, "vT": vT,
            "wq": chunked(Wq[:, sl]).astype(f16),
            "wk": chunked(Wk[:, sl]).astype(f16),
            "wv": chunked(Wv[:, sl]).astype(f16),
            "bq": bq[sl].reshape(64, 1).copy(),
            "bk": bk[sl].reshape(64, 1).copy(),
            "bv": bv[sl].reshape(64, 1).copy(),
            "wg": WG[h].astype(f16),                      # [64, 256]
            "bg": np.ascontiguousarray(
                bG[h].reshape(4, 64).T).astype(f32),      # [64, 4]
            "wfc": Wfc[h * 256:(h + 1) * 256, :].astype(f16),
        }
        maps.append(m)
    return maps


_compiled = None
last_results = None


def get_compiled():
    global _compiled
    if _compiled is None:
        _compiled = build_program()
    return _compiled


def kernel(**inputs):
    global last_results
    nc = get_compiled()
    in_maps = shard_inputs(inputs)
    last_results = bass_utils.run_bass_kernel_spmd(
        nc, in_maps, core_ids=list(range(8)))
    bfc = np.asarray(inputs["bfc"], np.float32)
    acc = np.zeros((N, E), np.float64)
    for res in last_results.results:
        acc += res["out"].astype(np.float64)
    full = (acc + bfc[None, :].astype(np.float64)).astype(np.float32)
    return full.reshape(1, N, E)


# revision 25
# speedup vs baseline: 1.2224x; 1.1718x over previous
"""CompoundHeadAttention TRN2 kernel (v2).

Full-input contract: kernel(**inputs) takes the unsharded tensors from
setup_inputs() and returns the full [1, 2048, 2048] float32 output.

Sharding (8 cores, tensor-parallel over the HK=8 kv heads):
  core h owns kv head h: its Wq/Wk/Wv column slice, its WG[h]/bG[h], and
  Wfc row-slice [h*256:(h+1)*256, :].  Each core computes its head's
  attention + its partial FC output [2048, 2048] in fp16; the host sums
  the 8 partials and adds bfc (the "all-reduce" of the row-sharded FC).

v2 device-side design (N=2048, E=2048, D=64, G=4 per core):
  - inputs qT/kT/vT [E, N] fp16 loaded as [128, 4, 1024] "quads", 4 per
    (tensor, window-pair); triggers spread over sync/gpsimd/vector queues
  - projections fp16, M=64 (no partition dup): psum [64, 512] per window
  - G transform fp16 from qt; qg0..3 stored as separate [64, 512] tiles
    at base partition 0 so ST needs no duplicated kt rows
  - ST fp16 [128 s, n] with exact 128-col causal trim (fp16 has no
    small-N matmul penalty); exp on ACT -> pt bf16; diagonal mask via
    DVE multiply with a precomputed triangular bf16 tile
  - PV bf16 accumulated [65, n] (row 64 = softmax denominators via a
    ones column in vo); PV lags ST by one chunk to hide ACT latency
  - normalize: reciprocal_approx_fast (DVE) + partition_broadcast (Pool)
    + tensor_mul -> hid fp16
  - FC fp16: hid [128, 128] stationary x wfc [128, 512]; psum->stage
    fp16 copies alternate DVE/Pool; out DMA per 128-row chunk
"""

import os
import sys

import numpy as np

if "/opt/trn_rl_repo" not in sys.path and os.path.isdir("/opt/trn_rl_repo"):
    sys.path.insert(0, "/opt/trn_rl_repo")

import concourse.bass as bass  # noqa: E402
import concourse.mybir as mybir  # noqa: E402
import concourse.tile as tile  # noqa: E402
from concourse import bacc  # noqa: E402
from concourse import bass_utils  # noqa: E402

F32 = mybir.dt.float32
F16 = mybir.dt.float16
BF16 = mybir.dt.bfloat16
AF = mybir.ActivationFunctionType

N = 2048
E = 2048
HK = 8
D = 64
G = 4
NB = 4         # 512-wide n-windows


def build_program():
    nc = bacc.Bacc("TRN2", target_bir_lowering=False, debug=False,
                   enable_asserts=False)

    # ---- DRAM I/O ----
    qT = nc.dram_tensor("qT", [E, N], F16, kind="ExternalInput").ap()
    kT = nc.dram_tensor("kT", [E, N], F16, kind="ExternalInput").ap()
    vT = nc.dram_tensor("vT", [E, N], F16, kind="ExternalInput").ap()
    # weight chunk layout: [128, 16*64] — e-chunk ec occupies cols [64ec, 64ec+64)
    wq = nc.dram_tensor("wq", [128, 16 * 128], F16, kind="ExternalInput").ap()
    wk = nc.dram_tensor("wk", [128, 16 * 128], F16, kind="ExternalInput").ap()
    wv = nc.dram_tensor("wv", [128, 16 * 64], F16, kind="ExternalInput").ap()
    bq = nc.dram_tensor("bq", [128, 1], F32, kind="ExternalInput").ap()
    bk = nc.dram_tensor("bk", [128, 1], F32, kind="ExternalInput").ap()
    bv = nc.dram_tensor("bv", [64, 1], F32, kind="ExternalInput").ap()
    wg = nc.dram_tensor("wg", [128, 256], F16, kind="ExternalInput").ap()
    bg = nc.dram_tensor("bg", [128, 2], F32, kind="ExternalInput").ap()
    wfc = nc.dram_tensor("wfc", [256, E], F16, kind="ExternalInput").ap()
    out = nc.dram_tensor("out", [N, E], F16, kind="ExternalOutput").ap()

    dumps = None
    if os.environ.get("KDUMP"):
        dumps = {
            "d_kt0": nc.dram_tensor("d_kt0", [64, 512], F16,
                                    kind="ExternalOutput").ap(),
            "d_qg00": nc.dram_tensor("d_qg00", [64, 512], F16,
                                     kind="ExternalOutput").ap(),
            "d_vo0": nc.dram_tensor("d_vo0", [128, 4, 65], BF16,
                                    kind="ExternalOutput").ap(),
            "d_hid01_0": nc.dram_tensor("d_hid01_0", [128, 512], F16,
                                        kind="ExternalOutput").ap(),
            "d_rec": nc.dram_tensor("d_rec", [64, 512], F32,
                                    kind="ExternalOutput").ap(),
        }

    with tile.TileContext(nc) as tc:
        build_tile_kernel(tc, qT=qT, kT=kT, vT=vT, wq=wq, wk=wk, wv=wv,
                          bq=bq, bk=bk, bv=bv, wg=wg, bg=bg, wfc=wfc,
                          out=out, dumps=dumps)
    nc.compile()
    return nc


def build_tile_kernel(tc, *, qT, kT, vT, wq, wk, wv, bq, bk, bv, wg, bg,
                      wfc, out, dumps=None):
    nc = tc.nc

    import contextlib
    ctx = contextlib.ExitStack()
    ctx.__enter__()
    cp = ctx.enter_context(tc.tile_pool(name="persist", bufs=1))

    def ptile(shape, dtype, name):
        return cp.tile(shape, dtype, tag=name, name=name)

    # ---- persistent constants in SBUF ----
    wq_sb = ptile([128, 16 * 128], F16, "wq_sb")
    wk_sb = ptile([128, 16 * 128], F16, "wk_sb")
    wv_sb = ptile([128, 16 * 64], F16, "wv_sb")
    wg_sb = ptile([128, 256], F16, "wg_sb")
    wfc0_sb = ptile([128, E], F16, "wfc0_sb")
    wfc1_sb = ptile([128, E], F16, "wfc1_sb")
    bq_sb = ptile([128, 1], F32, "bq_sb")
    bk_sb = ptile([128, 1], F32, "bk_sb")
    bv_sb = ptile([64, 1], F32, "bv_sb")
    bg_sb = ptile([128, 2], F32, "bg_sb")
    # causal mask constant: mask[s, n_local] = 1 if n_local >= s else 0,
    # duplicated side by side for the two g-halves of a pair
    mask_sb = ptile([128, 256], BF16, "mask_sb")
    nc.vector.memset(mask_sb[:], 1.0)
    # exp logit shift (cancels in softmax; keeps exp within fp16 range)
    eshift_sb = ptile([128, 1], F32, "eshift_sb")
    nc.vector.memset(eshift_sb[:], -35.0)
    mask3 = mask_sb[:].rearrange("p (h c) -> p h c", c=128)
    nc.gpsimd.affine_select(
        out=mask3, in_=mask3, compare_op=mybir.AluOpType.is_ge,
        fill=0.0, base=0, pattern=[[0, 2], [1, 128]], channel_multiplier=-1)

    # per-window persistent activations
    kt_w = [ptile([128, 512], F16, f"kt{j}") for j in range(NB)]
    # one tile per 128-token V chunk: DMA transpose writes at offset 0,
    # col 64 holds the ones column (softmax denominator row of PV)
    vo_w = [[ptile([128, 65], BF16, f"vo{j}_{c}") for c in range(4)]
            for j in range(NB)]
    for j in range(NB):
        for c in range(4):
            nc.vector.memset(vo_w[j][c][:, 64:65], 1.0)
    qg01_w = [ptile([128, 512], F16, f"qg01_{j}") for j in range(NB)]
    qg23_w = [ptile([128, 512], F16, f"qg23_{j}") for j in range(NB)]
    hid01_w = [ptile([128, 512], F16, f"hid01_{j}") for j in range(NB)]
    hid23_w = [ptile([128, 512], F16, f"hid23_{j}") for j in range(NB)]

    with ctx:
        in_pool = ctx.enter_context(tc.tile_pool(name="in_pool", bufs=5))
        qt_pool = ctx.enter_context(tc.tile_pool(name="qt_pool", bufs=2))
        vt_pool = ctx.enter_context(tc.tile_pool(name="vt_pool", bufs=2))
        pt_pool = ctx.enter_context(tc.tile_pool(name="pt_pool", bufs=3))
        rec_pool = ctx.enter_context(tc.tile_pool(name="rec_pool", bufs=2))
        stage_pool = ctx.enter_context(tc.tile_pool(name="stage", bufs=2))
        ps = ctx.enter_context(
            tc.tile_pool(name="ps", bufs=2, space="PSUM"))

        # quad tiles, filled by emit_dma, consumed by emit_proj
        quads = {t: [None] * 8 for t in "qkv"}

        def load_quad(t, idx, eng, src, P, qd):
            tl = in_pool.tile([128, 4, 1024], F16, tag=f"{t}quad",
                              name=f"{t}in_{idx}")
            quads[t][idx] = tl
            sl = src[qd * 512:(qd + 1) * 512,
                     bass.ds(P * 1024, 1024)]
            eng.dma_start(tl[:], sl.rearrange("(e p) c -> p e c", p=128))

        def emit_dma_head():
            """wq + pair-0 input quads + early consts (phase A)."""
            nc.sync.dma_start(wq_sb[:], wq[:])
            for qd in range(4):
                load_quad("q", qd, nc.sync, qT, 0, qd)
                yield
            nc.sync.dma_start(wk_sb[:], wk[:])
            for qd in range(4):
                load_quad("k", qd, nc.gpsimd, kT, 0, qd)
                yield
            nc.sync.dma_start(wv_sb[:], wv[:])
            for qd in range(4):
                load_quad("v", qd, nc.gpsimd, vT, 0, qd)
                yield
            for dst, src in ((bq_sb, bq), (bk_sb, bk), (bv_sb, bv),
                             (wg_sb, wg), (bg_sb, bg)):
                nc.sync.dma_start(dst[:], src[:])
            yield

        def emit_dma_tail():
            """pair-1 input quads + wfc (phase B, overlapped)."""
            for qd in range(4):
                load_quad("q", 4 + qd, nc.sync, qT, 1, qd)
                yield
            for qd in range(4):
                load_quad("k", 4 + qd, nc.gpsimd, kT, 1, qd)
                yield
            for qd in range(4):
                load_quad("v", 4 + qd, nc.gpsimd, vT, 1, qd)
                yield
            nc.gpsimd.dma_start(wfc0_sb[:], wfc[0:128, :])
            yield
            nc.gpsimd.dma_start(wfc1_sb[:], wfc[128:256, :])
            yield

        dma_gens = {}

        def ensure_quad(t, idx):
            gen = dma_gens[0] if idx < 4 else dma_gens[1]
            while quads[t][idx] is None:
                try:
                    next(gen)
                except StopIteration:
                    break
            assert quads[t][idx] is not None

        def emit_proj(P):
            """projections + G + V transpose for window pair P."""
            wins = (2 * P, 2 * P + 1)

            # --- Q (dup'd 2x64 output partitions) ---
            q0_ps = ps.tile([128, 512], F32, tag="mm", name="q0_ps")
            q1_ps = ps.tile([128, 512], F32, tag="mm", name="q1_ps")
            for ec in range(16):
                ensure_quad("q", P * 4 + ec // 4)
                quad = quads["q"][P * 4 + ec // 4]
                w = wq_sb[:, bass.ts(ec, 128)]
                nc.tensor.matmul(q0_ps[:], w, quad[:, ec % 4, 0:512],
                                 start=(ec == 0), stop=(ec == 15))
                yield
                nc.tensor.matmul(q1_ps[:], w, quad[:, ec % 4, 512:1024],
                                 start=(ec == 0), stop=(ec == 15))
                yield
            qt0 = qt_pool.tile([128, 512], F16, tag="qt", name="qt0")
            qt1 = qt_pool.tile([128, 512], F16, tag="qt", name="qt1")
            nc.scalar.activation(qt0[:], q0_ps[:], AF.Identity, bias=bq_sb[:])
            nc.scalar.activation(qt1[:], q1_ps[:], AF.Identity, bias=bq_sb[:])
            # --- G transform (row-tiled pair01 / pair23) ---
            for wi, qt in ((wins[0], qt0), (wins[1], qt1)):
                g01_ps = ps.tile([128, 512], F32, tag="mm", name="g01_ps")
                nc.tensor.matmul(g01_ps[:], wg_sb[0:64, 0:128], qt[0:64, :],
                                 start=True, stop=True)
                yield
                g23_ps = ps.tile([128, 512], F32, tag="mm", name="g23_ps")
                nc.tensor.matmul(g23_ps[:], wg_sb[64:128, 128:256],
                                 qt[64:128, :], start=True, stop=True)
                yield
                nc.scalar.activation(qg01_w[wi][:], g01_ps[:], AF.Identity,
                                     bias=bg_sb[:, 0:1])
                nc.scalar.activation(qg23_w[wi][:], g23_ps[:], AF.Identity,
                                     bias=bg_sb[:, 1:2])

            # --- K (dup'd) ---
            k0_ps = ps.tile([128, 512], F32, tag="mm", name="k0_ps")
            k1_ps = ps.tile([128, 512], F32, tag="mm", name="k1_ps")
            for ec in range(16):
                ensure_quad("k", P * 4 + ec // 4)
                quad = quads["k"][P * 4 + ec // 4]
                w = wk_sb[:, bass.ts(ec, 128)]
                nc.tensor.matmul(k0_ps[:], w, quad[:, ec % 4, 0:512],
                                 start=(ec == 0), stop=(ec == 15))
                yield
                nc.tensor.matmul(k1_ps[:], w, quad[:, ec % 4, 512:1024],
                                 start=(ec == 0), stop=(ec == 15))
                yield
            nc.scalar.activation(kt_w[wins[0]][:], k0_ps[:], AF.Identity,
                                 bias=bk_sb[:])
            nc.scalar.activation(kt_w[wins[1]][:], k1_ps[:], AF.Identity,
                                 bias=bk_sb[:])

            # --- V ---
            v0_ps = ps.tile([64, 512], F32, tag="mm", name="v0_ps")
            v1_ps = ps.tile([64, 512], F32, tag="mm", name="v1_ps")
            for ec in range(16):
                ensure_quad("v", P * 4 + ec // 4)
                quad = quads["v"][P * 4 + ec // 4]
                w = wv_sb[:, bass.ts(ec, 64)]
                nc.tensor.matmul(v0_ps[:], w, quad[:, ec % 4, 0:512],
                                 start=(ec == 0), stop=(ec == 15))
                yield
                nc.tensor.matmul(v1_ps[:], w, quad[:, ec % 4, 512:1024],
                                 start=(ec == 0), stop=(ec == 15))
                yield
            for wi, v_ps in ((wins[0], v0_ps), (wins[1], v1_ps)):
                vt_sb = vt_pool.tile([64, 512], BF16, tag="vt", name="vt_sb")
                nc.scalar.activation(vt_sb[:], v_ps[:], AF.Identity,
                                     bias=bv_sb[:])
                for c in range(4):
                    nc.sync.dma_start_transpose(
                        vo_w[wi][c][:, 0:64],
                        vt_sb[:, bass.ts(c, 128)])
                yield

        def emit_attn(j):
            klast = 4 * j + 3
            for p in range(2):  # g-pairs (2p, 2p+1)
                pv_a = ps.tile([65, 512], F32, tag="pv", name="pv_a")
                pv_b = ps.tile([65, 512], F32, tag="pv", name="pv_b")
                pending = None

                def flush(pend):
                    pt, k, off = pend
                    vsl = vo_w[k // 4][k % 4][:, 0:65]
                    nc.tensor.matmul(pv_a[:, off:512], vsl, pt[:, off:512],
                                     start=(k == 0), stop=(k == klast))
                    nc.tensor.matmul(pv_b[:, off:512], vsl,
                                     pt[:, 512 + off:1024],
                                     start=(k == 0), stop=(k == klast))

                qg = qg01_w[j] if p == 0 else qg23_w[j]
                for k in range(klast + 1):
                    kc = kt_w[k // 4][:, bass.ts(k % 4, 128)]
                    i = k - 4 * j
                    off = max(0, 128 * i)
                    st = ps.tile([128, 1024], F32, tag="st", name="st")
                    nc.tensor.matmul(st[:, off:512], kc[0:64, :],
                                     qg[0:64, off:512],
                                     start=True, stop=True)
                    nc.tensor.matmul(st[:, 512 + off:1024], kc[64:128, :],
                                     qg[64:128, off:512],
                                     start=True, stop=True)
                    yield
                    pt = pt_pool.tile([128, 1024], BF16, tag="pt", name="pt")
                    st3 = st[:].rearrange("p (h c) -> p h c", c=512)
                    pt3 = pt[:].rearrange("p (h c) -> p h c", c=512)
                    # exp(8S - 35): the shift cancels in the softmax ratio
                    # and keeps all exp outputs within fp16/bf16 range (HW
                    # ACT saturates 16-bit outputs near the fp16 max)
                    nc.scalar.activation(pt3[:, :, off:512],
                                         st3[:, :, off:512],
                                         AF.Exp, scale=8.0,
                                         bias=eshift_sb[:])
                    if i >= 0:
                        # zero out below-diagonal cols [off, off+128)
                        nc.vector.tensor_mul(pt3[:, :, off:off + 128],
                                             pt3[:, :, off:off + 128],
                                             mask3)
                    if pending is not None:
                        flush(pending)
                        yield
                    pending = (pt, k, off)
                flush(pending)
                yield
                # normalize: hid[half] = pv[0:64] * 1/pv[64]
                hid = hid01_w[j] if p == 0 else hid23_w[j]
                for half, pv in ((0, pv_a), (1, pv_b)):
                    # custom-DVE recip can't read PSUM on HW: stage to SBUF
                    den = rec_pool.tile([1, 512], F32, tag="den", name="den")
                    nc.vector.tensor_copy(den[:], pv[64:65, :])
                    rec = rec_pool.tile([1, 512], F32, tag="rec", name="rec")
                    nc.vector.reciprocal_approx_fast(rec[:], den[:])
                    recr = rec_pool.tile([64, 512], F32, tag="recr",
                                         name="recr")
                    nc.gpsimd.partition_broadcast(recr[:], rec[:])
                    if dumps is not None and j == 0 and p == 0 and half == 0:
                        nc.sync.dma_start(dumps["d_rec"][:], recr[:])
                    nc.vector.tensor_mul(hid[half * 64:half * 64 + 64, :],
                                         pv[0:64, :], recr[:])

        def emit_fc(j):
            for m in range(4):
                msl = bass.ts(m, 128)
                stage = stage_pool.tile([128, 2048], F16, tag="fco",
                                        name="stage")
                for eo in range(4):
                    fc_ps = ps.tile([128, 512], F32, tag="mm",
                                    name="fc_ps")
                    nc.tensor.matmul(fc_ps[:], hid01_w[j][:, msl],
                                     wfc0_sb[:, bass.ts(eo, 512)],
                                     start=True, stop=False)
                    yield
                    nc.tensor.matmul(fc_ps[:], hid23_w[j][:, msl],
                                     wfc1_sb[:, bass.ts(eo, 512)],
                                     start=False, stop=True)
                    yield
                    nc.vector.tensor_copy(stage[:, bass.ts(eo, 512)],
                                          fc_ps[:])
                nc.sync.dma_start(
                    out[512 * j + 128 * m: 512 * j + 128 * m + 128, :],
                    stage[:])

        from itertools import chain as ichain

        def drain(g):
            for _ in g:
                pass

        def rr(pairs):
            """round-robin emission: [(generator, steps_per_turn)]"""
            live = [[g, w] for g, w in pairs]
            while live:
                for gw in list(live):
                    g, w = gw
                    try:
                        for _ in range(w):
                            next(g)
                    except StopIteration:
                        live.remove(gw)

        # Phase A: DMA ramp + pair-0 projections
        dma_gens[0] = emit_dma_head()
        dma_gens[1] = emit_dma_tail()
        rr([(dma_gens[0], 2), (emit_proj(0), 3)])
        # Phase B: window-0/1 attention + pair-1 projections + tail DMAs
        rr([(ichain(emit_attn(0), emit_attn(1)), 1),
            (emit_proj(1), 2),
            (dma_gens[1], 1)])
        # Later: attention with FC of completed windows as PE filler
        rr([(emit_attn(2), 3), (emit_fc(0), 2)])
        rr([(emit_attn(3), 1), (ichain(emit_fc(1), emit_fc(2)), 1)])
        drain(emit_fc(3))

        if dumps is not None:
            nc.sync.dma_start(dumps["d_kt0"][:], kt_w[0][0:64, :])
            nc.sync.dma_start(dumps["d_qg00"][:], qg01_w[0][0:64, :])
            for c in range(4):
                nc.sync.dma_start(dumps["d_vo0"][:, c, :], vo_w[0][c][:, :])
            nc.sync.dma_start(dumps["d_hid01_0"][:], hid01_w[0][:, :])


def shard_inputs(inputs):
    """full inputs -> list of 8 per-core in_maps (numpy, device layouts)"""
    f16 = np.float16
    f32 = np.float32
    q = np.asarray(inputs["q"], f32)[0]
    k = np.asarray(inputs["k"], f32)[0]
    v = np.asarray(inputs["v"], f32)[0]
    Wq = np.asarray(inputs["Wq"], f32)
    Wk = np.asarray(inputs["Wk"], f32)
    Wv = np.asarray(inputs["Wv"], f32)
    bq = np.asarray(inputs["bq"], f32)
    bk = np.asarray(inputs["bk"], f32)
    bv = np.asarray(inputs["bv"], f32)
    WG = np.asarray(inputs["WG"], f32)
    bG = np.asarray(inputs["bG"], f32)
    Wfc = np.asarray(inputs["Wfc"], f32)

    qT = np.ascontiguousarray(q.T.astype(f16))
    kT = np.ascontiguousarray(k.T.astype(f16))
    vT = np.ascontiguousarray(v.T.astype(f16))

    def chunked(w):
        # [E, 64] -> [128, 16*64]: e-chunk ec at cols [64ec, 64ec+64)
        M = w.shape[1]
        return np.ascontiguousarray(
            w.reshape(16, 128, M).transpose(1, 0, 2).reshape(128, 16 * M))

    maps = []
    for h in range(HK):
        sl = slice(h * D, (h + 1) * D)
        wq_h = Wq[:, sl]
        wk_h = Wk[:, sl]
        m = {
            "qT": qT, "kT": kT, "vT": vT,
            "wq": chunked(np.concatenate([wq_h, wq_h], 1)).astype(f16),
            "wk": chunked(np.concatenate([wk_h, wk_h], 1)).astype(f16),
            "wv": chunked(Wv[:, sl]).astype(f16),
            "bq": np.concatenate([bq[sl], bq[sl]]).reshape(128, 1).copy(),
            "bk": np.concatenate([bk[sl], bk[sl]]).reshape(128, 1).copy(),
            "bv": bv[sl].reshape(64, 1).copy(),
            "wg": np.concatenate([WG[h], WG[h]], 0).astype(f16),  # [128, 256]
            "bg": np.ascontiguousarray(
                bG[h].reshape(2, 128).T).astype(f32),     # [128, 2]
            "wfc": Wfc[h * 256:(h + 1) * 256, :].astype(f16),
        }
        maps.append(m)
    return maps


_compiled = None
last_results = None


def get_compiled():
    global _compiled
    if _compiled is None:
        _compiled = build_program()
    return _compiled


def kernel(**inputs):
    global last_results
    nc = get_compiled()
    in_maps = shard_inputs(inputs)
    last_results = bass_utils.run_bass_kernel_spmd(
        nc, in_maps, core_ids=list(range(8)))
    bfc = np.asarray(inputs["bfc"], np.float32)
    acc = np.zeros((N, E), np.float64)
    for res in last_results.results:
        acc += res["out"].astype(np.float64)
    full = (acc + bfc[None, :].astype(np.float64)).astype(np.float32)
    return full.reshape(1, N, E)


# revision 28
# speedup vs baseline: 1.2566x; 1.0280x over previous
"""CompoundHeadAttention TRN2 kernel (v2).

Full-input contract: kernel(**inputs) takes the unsharded tensors from
setup_inputs() and returns the full [1, 2048, 2048] float32 output.

Sharding (8 cores, tensor-parallel over the HK=8 kv heads):
  core h owns kv head h: its Wq/Wk/Wv column slice, its WG[h]/bG[h], and
  Wfc row-slice [h*256:(h+1)*256, :].  Each core computes its head's
  attention + its partial FC output [2048, 2048] in fp16; the host sums
  the 8 partials and adds bfc (the "all-reduce" of the row-sharded FC).

v2 device-side design (N=2048, E=2048, D=64, G=4 per core):
  - inputs qT/kT/vT [E, N] fp16 loaded as [128, 4, 1024] "quads", 4 per
    (tensor, window-pair); triggers spread over sync/gpsimd/vector queues
  - projections fp16, M=64 (no partition dup): psum [64, 512] per window
  - G transform fp16 from qt; qg0..3 stored as separate [64, 512] tiles
    at base partition 0 so ST needs no duplicated kt rows
  - ST fp16 [128 s, n] with exact 128-col causal trim (fp16 has no
    small-N matmul penalty); exp on ACT -> pt bf16; diagonal mask via
    DVE multiply with a precomputed triangular bf16 tile
  - PV bf16 accumulated [65, n] (row 64 = softmax denominators via a
    ones column in vo); PV lags ST by one chunk to hide ACT latency
  - normalize: reciprocal_approx_fast (DVE) + partition_broadcast (Pool)
    + tensor_mul -> hid fp16
  - FC fp16: hid [128, 128] stationary x wfc [128, 512]; psum->stage
    fp16 copies alternate DVE/Pool; out DMA per 128-row chunk
"""

import os
import sys

import numpy as np

if "/opt/trn_rl_repo" not in sys.path and os.path.isdir("/opt/trn_rl_repo"):
    sys.path.insert(0, "/opt/trn_rl_repo")

import concourse.bass as bass  # noqa: E402
import concourse.mybir as mybir  # noqa: E402
import concourse.tile as tile  # noqa: E402
from concourse import bacc  # noqa: E402
from concourse import bass_utils  # noqa: E402

F32 = mybir.dt.float32
F16 = mybir.dt.float16
BF16 = mybir.dt.bfloat16
AF = mybir.ActivationFunctionType

N = 2048
E = 2048
HK = 8
D = 64
G = 4
NB = 4         # 512-wide n-windows


def build_program():
    nc = bacc.Bacc("TRN2", target_bir_lowering=False, debug=False,
                   enable_asserts=False)

    # ---- DRAM I/O ----
    # pre-swizzled input layout: [p, qd, P, e, c] = x^T[qd*512+e*128+p,
    # P*1024+c] so one quad = 128 contiguous 8 KB runs (1 descriptor per
    # partition)
    qT = nc.dram_tensor("qT", [128, 4, 2, 4, 1024], F16,
                        kind="ExternalInput").ap()
    kT = nc.dram_tensor("kT", [128, 4, 2, 4, 1024], F16,
                        kind="ExternalInput").ap()
    vT = nc.dram_tensor("vT", [128, 4, 2, 4, 1024], F16,
                        kind="ExternalInput").ap()
    # weight chunk layout: [128, 16*64] — e-chunk ec occupies cols [64ec, 64ec+64)
    wq = nc.dram_tensor("wq", [128, 16 * 128], F16, kind="ExternalInput").ap()
    wk = nc.dram_tensor("wk", [128, 16 * 128], F16, kind="ExternalInput").ap()
    wv = nc.dram_tensor("wv", [128, 16 * 64], F16, kind="ExternalInput").ap()
    bq = nc.dram_tensor("bq", [128, 1], F32, kind="ExternalInput").ap()
    bk = nc.dram_tensor("bk", [128, 1], F32, kind="ExternalInput").ap()
    bv = nc.dram_tensor("bv", [64, 1], F32, kind="ExternalInput").ap()
    wg = nc.dram_tensor("wg", [128, 256], F16, kind="ExternalInput").ap()
    bg = nc.dram_tensor("bg", [128, 2], F32, kind="ExternalInput").ap()
    wfc = nc.dram_tensor("wfc", [256, E], F16, kind="ExternalInput").ap()
    out = nc.dram_tensor("out", [N, E], F16, kind="ExternalOutput").ap()

    dumps = None
    if os.environ.get("KDUMP"):
        dumps = {
            "d_kt0": nc.dram_tensor("d_kt0", [64, 512], F16,
                                    kind="ExternalOutput").ap(),
            "d_qg00": nc.dram_tensor("d_qg00", [64, 512], F16,
                                     kind="ExternalOutput").ap(),
            "d_vo0": nc.dram_tensor("d_vo0", [128, 4, 65], BF16,
                                    kind="ExternalOutput").ap(),
            "d_hid01_0": nc.dram_tensor("d_hid01_0", [128, 512], F16,
                                        kind="ExternalOutput").ap(),
            "d_rec": nc.dram_tensor("d_rec", [64, 512], F32,
                                    kind="ExternalOutput").ap(),
        }

    with tile.TileContext(nc) as tc:
        build_tile_kernel(tc, qT=qT, kT=kT, vT=vT, wq=wq, wk=wk, wv=wv,
                          bq=bq, bk=bk, bv=bv, wg=wg, bg=bg, wfc=wfc,
                          out=out, dumps=dumps)
    nc.compile()
    return nc


def build_tile_kernel(tc, *, qT, kT, vT, wq, wk, wv, bq, bk, bv, wg, bg,
                      wfc, out, dumps=None):
    nc = tc.nc

    import contextlib
    ctx = contextlib.ExitStack()
    ctx.__enter__()
    cp = ctx.enter_context(tc.tile_pool(name="persist", bufs=1))

    def ptile(shape, dtype, name):
        return cp.tile(shape, dtype, tag=name, name=name)

    # ---- persistent constants in SBUF ----
    wq_sb = ptile([128, 16 * 128], F16, "wq_sb")
    wk_sb = ptile([128, 16 * 128], F16, "wk_sb")
    wv_sb = ptile([128, 16 * 64], F16, "wv_sb")
    wg_sb = ptile([128, 256], F16, "wg_sb")
    wfc0_sb = ptile([128, E], F16, "wfc0_sb")
    wfc1_sb = ptile([128, E], F16, "wfc1_sb")
    bq_sb = ptile([128, 1], F32, "bq_sb")
    bk_sb = ptile([128, 1], F32, "bk_sb")
    bv_sb = ptile([64, 1], F32, "bv_sb")
    bg_sb = ptile([128, 2], F32, "bg_sb")
    # causal mask constant: mask[s, n_local] = 1 if n_local >= s else 0,
    # duplicated side by side for the two g-halves of a pair
    mask_sb = ptile([128, 256], BF16, "mask_sb")
    nc.vector.memset(mask_sb[:], 1.0)
    # exp logit shift (cancels in softmax; keeps exp within fp16 range)
    eshift_sb = ptile([128, 1], F32, "eshift_sb")
    nc.vector.memset(eshift_sb[:], -35.0)
    mask3 = mask_sb[:].rearrange("p (h c) -> p h c", c=128)
    nc.gpsimd.affine_select(
        out=mask3, in_=mask3, compare_op=mybir.AluOpType.is_ge,
        fill=0.0, base=0, pattern=[[0, 2], [1, 128]], channel_multiplier=-1)
    # dummy broadcast: preloads the gpsimd pool config for
    # partition_broadcast off the critical path
    warm_sb = ptile([64, 1], F32, "warm_sb")
    nc.gpsimd.partition_broadcast(warm_sb[:], eshift_sb[0:1, 0:1])

    # per-window persistent activations
    kt_w = [ptile([128, 512], F16, f"kt{j}") for j in range(NB)]
    # one tile per 128-token V chunk: DMA transpose writes at offset 0,
    # col 64 holds the ones column (softmax denominator row of PV)
    vo_w = [[ptile([128, 65], BF16, f"vo{j}_{c}") for c in range(4)]
            for j in range(NB)]
    for j in range(NB):
        for c in range(4):
            nc.vector.memset(vo_w[j][c][:, 64:65], 1.0)
    qg01_w = [ptile([128, 512], F16, f"qg01_{j}") for j in range(NB)]
    qg23_w = [ptile([128, 512], F16, f"qg23_{j}") for j in range(NB)]
    hid01_w = [ptile([128, 512], F16, f"hid01_{j}") for j in range(NB)]
    hid23_w = [ptile([128, 512], F16, f"hid23_{j}") for j in range(NB)]

    with ctx:
        in_pool = ctx.enter_context(tc.tile_pool(name="in_pool", bufs=5))
        qt_pool = ctx.enter_context(tc.tile_pool(name="qt_pool", bufs=2))
        vt_pool = ctx.enter_context(tc.tile_pool(name="vt_pool", bufs=2))
        pt_pool = ctx.enter_context(tc.tile_pool(name="pt_pool", bufs=3))
        rec_pool = ctx.enter_context(tc.tile_pool(name="rec_pool", bufs=2))
        stage_pool = ctx.enter_context(tc.tile_pool(name="stage", bufs=2))
        ps = ctx.enter_context(
            tc.tile_pool(name="ps", bufs=2, space="PSUM"))

        # quad tiles, filled by emit_dma, consumed by emit_proj
        quads = {t: [None] * 8 for t in "qkv"}

        def load_quad(t, idx, eng, src, P, qd):
            tl = in_pool.tile([128, 4, 1024], F16, tag=f"{t}quad",
                              name=f"{t}in_{idx}")
            quads[t][idx] = tl
            eng.dma_start(tl[:], src[:, qd, P])

        def emit_dma_head():
            """weights + pair-0 input quads + early consts (phase A)."""
            nc.sync.dma_start(wq_sb[:], wq[:])
            nc.sync.dma_start(wk_sb[:], wk[:])
            nc.sync.dma_start(wv_sb[:], wv[:])
            for qd in range(4):
                load_quad("q", qd, nc.gpsimd, qT, 0, qd)
                load_quad("k", qd, nc.gpsimd, kT, 0, qd)
                load_quad("v", qd, nc.gpsimd, vT, 0, qd)
                yield
            for dst, srcap in ((bq_sb, bq), (bk_sb, bk), (bv_sb, bv),
                               (wg_sb, wg), (bg_sb, bg)):
                nc.sync.dma_start(dst[:], srcap[:])
            yield

        def emit_dma_tail():
            """pair-1 input quads + wfc (phase B, overlapped)."""
            for qd in range(4):
                load_quad("q", 4 + qd, nc.gpsimd, qT, 1, qd)
                load_quad("k", 4 + qd, nc.gpsimd, kT, 1, qd)
                load_quad("v", 4 + qd, nc.gpsimd, vT, 1, qd)
                yield
            nc.sync.dma_start(wfc0_sb[:], wfc[0:128, :])
            yield
            nc.sync.dma_start(wfc1_sb[:], wfc[128:256, :])
            yield

        dma_gens = {}

        def ensure_quad(t, idx):
            gen = dma_gens[0] if idx < 4 else dma_gens[1]
            while quads[t][idx] is None:
                try:
                    next(gen)
                except StopIteration:
                    break
            assert quads[t][idx] is not None

        def emit_proj(P):
            """projections + G + V transpose for window pair P."""
            wins = (2 * P, 2 * P + 1)

            # Q/K psum pairs live in wide "st" tiles (attention is not
            # running during projections); V pair + G use the "mm" tag.
            q_ps = ps.tile([128, 1024], F32, tag="st", name="q_ps")
            k_ps = ps.tile([128, 1024], F32, tag="st", name="k_ps")
            v0_ps = ps.tile([64, 512], F32, tag="mm", name="v0_ps")
            v1_ps = ps.tile([64, 512], F32, tag="mm", name="v1_ps")
            for ec in range(16):
                qi = P * 4 + ec // 4
                for t, w_sb, dsts, mw in (
                        ("q", wq_sb, (q_ps[:, 0:512], q_ps[:, 512:1024]),
                         128),
                        ("k", wk_sb, (k_ps[:, 0:512], k_ps[:, 512:1024]),
                         128),
                        ("v", wv_sb, (v0_ps[:], v1_ps[:]), 64)):
                    ensure_quad(t, qi)
                    quad = quads[t][qi]
                    w = w_sb[:, bass.ts(ec, mw)]
                    nc.tensor.matmul(dsts[0], w, quad[:, ec % 4, 0:512],
                                     start=(ec == 0), stop=(ec == 15))
                    nc.tensor.matmul(dsts[1], w, quad[:, ec % 4, 512:1024],
                                     start=(ec == 0), stop=(ec == 15))
                    yield
            qt0 = qt_pool.tile([128, 512], F16, tag="qt", name="qt0")
            qt1 = qt_pool.tile([128, 512], F16, tag="qt", name="qt1")
            nc.scalar.activation(qt0[:], q_ps[:, 0:512], AF.Identity,
                                 bias=bq_sb[:])
            nc.scalar.activation(qt1[:], q_ps[:, 512:1024], AF.Identity,
                                 bias=bq_sb[:])
            nc.scalar.activation(kt_w[wins[0]][:], k_ps[:, 0:512],
                                 AF.Identity, bias=bk_sb[:])
            nc.scalar.activation(kt_w[wins[1]][:], k_ps[:, 512:1024],
                                 AF.Identity, bias=bk_sb[:])
            for wi, vsl in ((wins[0], v0_ps[:]),
                            (wins[1], v1_ps[:])):
                vt_sb = vt_pool.tile([64, 512], BF16, tag="vt", name="vt_sb")
                nc.scalar.activation(vt_sb[:], vsl, AF.Identity,
                                     bias=bv_sb[:])
                for c in range(4):
                    nc.sync.dma_start_transpose(
                        vo_w[wi][c][:, 0:64],
                        vt_sb[:, bass.ts(c, 128)])
            # --- G transform (row-tiled pair01 / pair23) ---
            for wi, qt in ((wins[0], qt0), (wins[1], qt1)):
                g01_ps = ps.tile([128, 512], F32, tag="mm", name="g01_ps")
                nc.tensor.matmul(g01_ps[:], wg_sb[0:64, 0:128], qt[0:64, :],
                                 start=True, stop=True)
                yield
                g23_ps = ps.tile([128, 512], F32, tag="mm", name="g23_ps")
                nc.tensor.matmul(g23_ps[:], wg_sb[64:128, 128:256],
                                 qt[64:128, :], start=True, stop=True)
                yield
                nc.scalar.activation(qg01_w[wi][:], g01_ps[:], AF.Identity,
                                     bias=bg_sb[:, 0:1])
                nc.scalar.activation(qg23_w[wi][:], g23_ps[:], AF.Identity,
                                     bias=bg_sb[:, 1:2])

        def emit_attn(j):
            klast = 4 * j + 3
            for p in range(2):  # g-pairs (2p, 2p+1)
                pv_a = ps.tile([65, 512], F32, tag="pv", name="pv_a")
                pv_b = ps.tile([65, 512], F32, tag="pv", name="pv_b")
                pending = None

                def flush(pend):
                    pt, k, off = pend
                    vsl = vo_w[k // 4][k % 4][:, 0:65]
                    nc.tensor.matmul(pv_a[:, off:512], vsl, pt[:, off:512],
                                     start=(k == 0), stop=(k == klast))
                    nc.tensor.matmul(pv_b[:, off:512], vsl,
                                     pt[:, 512 + off:1024],
                                     start=(k == 0), stop=(k == klast))

                qg = qg01_w[j] if p == 0 else qg23_w[j]
                for k in range(klast + 1):
                    kc = kt_w[k // 4][:, bass.ts(k % 4, 128)]
                    i = k - 4 * j
                    off = max(0, 128 * i)
                    st = ps.tile([128, 1024], F32, tag="st", name="st")
                    nc.tensor.matmul(st[:, off:512], kc[0:64, :],
                                     qg[0:64, off:512],
                                     start=True, stop=True)
                    nc.tensor.matmul(st[:, 512 + off:1024], kc[64:128, :],
                                     qg[64:128, off:512],
                                     start=True, stop=True)
                    yield
                    pt = pt_pool.tile([128, 1024], BF16, tag="pt", name="pt")
                    st3 = st[:].rearrange("p (h c) -> p h c", c=512)
                    pt3 = pt[:].rearrange("p (h c) -> p h c", c=512)
                    # exp(8S - 35): the shift cancels in the softmax ratio
                    # and keeps all exp outputs within fp16/bf16 range (HW
                    # ACT saturates 16-bit outputs near the fp16 max)
                    nc.scalar.activation(pt3[:, :, off:512],
                                         st3[:, :, off:512],
                                         AF.Exp, scale=8.0,
                                         bias=eshift_sb[:])
                    if i >= 0:
                        # zero out below-diagonal cols [off, off+128)
                        nc.vector.tensor_mul(pt3[:, :, off:off + 128],
                                             pt3[:, :, off:off + 128],
                                             mask3)
                    if pending is not None:
                        flush(pending)
                        yield
                    pending = (pt, k, off)
                flush(pending)
                yield
                # normalize: hid[half] = pv[0:64] * 1/pv[64]
                hid = hid01_w[j] if p == 0 else hid23_w[j]
                for half, pv in ((0, pv_a), (1, pv_b)):
                    # custom-DVE recip can't read PSUM on HW: stage to SBUF
                    den = rec_pool.tile([1, 512], F32, tag="den", name="den")
                    nc.vector.tensor_copy(den[:], pv[64:65, :])
                    rec = rec_pool.tile([1, 512], F32, tag="rec", name="rec")
                    nc.vector.reciprocal_approx_fast(rec[:], den[:])
                    recr = rec_pool.tile([64, 512], F32, tag="recr",
                                         name="recr")
                    nc.gpsimd.partition_broadcast(recr[:], rec[:])
                    if dumps is not None and j == 0 and p == 0 and half == 0:
                        nc.sync.dma_start(dumps["d_rec"][:], recr[:])
                    nc.vector.tensor_mul(hid[half * 64:half * 64 + 64, :],
                                         pv[0:64, :], recr[:])

        def emit_fc(j):
            for m in range(4):
                msl = bass.ts(m, 128)
                stage = stage_pool.tile([128, 2048], F16, tag="fco",
                                        name="stage")
                for eo in range(4):
                    fc_ps = ps.tile([128, 512], F32, tag="mm",
                                    name="fc_ps")
                    nc.tensor.matmul(fc_ps[:], hid01_w[j][:, msl],
                                     wfc0_sb[:, bass.ts(eo, 512)],
                                     start=True, stop=False)
                    yield
                    nc.tensor.matmul(fc_ps[:], hid23_w[j][:, msl],
                                     wfc1_sb[:, bass.ts(eo, 512)],
                                     start=False, stop=True)
                    yield
                    nc.vector.tensor_copy(stage[:, bass.ts(eo, 512)],
                                          fc_ps[:])
                nc.sync.dma_start(
                    out[512 * j + 128 * m: 512 * j + 128 * m + 128, :],
                    stage[:])

        from itertools import chain as ichain

        def drain(g):
            for _ in g:
                pass

        def rr(pairs):
            """round-robin emission: [(generator, steps_per_turn)]"""
            live = [[g, w] for g, w in pairs]
            while live:
                for gw in list(live):
                    g, w = gw
                    try:
                        for _ in range(w):
                            next(g)
                    except StopIteration:
                        live.remove(gw)

        # Phase A: DMA ramp + pair-0 projections
        dma_gens[0] = emit_dma_head()
        dma_gens[1] = emit_dma_tail()
        rr([(dma_gens[0], 2), (emit_proj(0), 3)])
        # Phase B: window-0/1 attention + pair-1 projections + tail DMAs
        rr([(ichain(emit_attn(0), emit_attn(1)), 1),
            (emit_proj(1), 2),
            (dma_gens[1], 1)])
        # Later: attention with FC of completed windows as PE filler
        rr([(emit_attn(2), 3), (emit_fc(0), 2)])
        rr([(emit_attn(3), 1), (ichain(emit_fc(1), emit_fc(2)), 1)])
        drain(emit_fc(3))

        if dumps is not None:
            nc.sync.dma_start(dumps["d_kt0"][:], kt_w[0][0:64, :])
            nc.sync.dma_start(dumps["d_qg00"][:], qg01_w[0][0:64, :])
            for c in range(4):
                nc.sync.dma_start(dumps["d_vo0"][:, c, :], vo_w[0][c][:, :])
            nc.sync.dma_start(dumps["d_hid01_0"][:], hid01_w[0][:, :])


def shard_inputs(inputs):
    """full inputs -> list of 8 per-core in_maps (numpy, device layouts)"""
    f16 = np.float16
    f32 = np.float32
    q = np.asarray(inputs["q"], f32)[0]
    k = np.asarray(inputs["k"], f32)[0]
    v = np.asarray(inputs["v"], f32)[0]
    Wq = np.asarray(inputs["Wq"], f32)
    Wk = np.asarray(inputs["Wk"], f32)
    Wv = np.asarray(inputs["Wv"], f32)
    bq = np.asarray(inputs["bq"], f32)
    bk = np.asarray(inputs["bk"], f32)
    bv = np.asarray(inputs["bv"], f32)
    WG = np.asarray(inputs["WG"], f32)
    bG = np.asarray(inputs["bG"], f32)
    Wfc = np.asarray(inputs["Wfc"], f32)

    def swizzle(x):
        # x [N, E] -> xT [E, N] -> [p, qd, P, e, c]
        xt = x.T.astype(f16).reshape(4, 4, 128, 2, 1024)   # (qd, e, p, P, c)
        return np.ascontiguousarray(xt.transpose(2, 0, 3, 1, 4))

    qT = swizzle(q)
    kT = swizzle(k)
    vT = swizzle(v)

    def chunked(w):
        # [E, 64] -> [128, 16*64]: e-chunk ec at cols [64ec, 64ec+64)
        M = w.shape[1]
        return np.ascontiguousarray(
            w.reshape(16, 128, M).transpose(1, 0, 2).reshape(128, 16 * M))

    maps = []
    for h in range(HK):
        sl = slice(h * D, (h + 1) * D)
        wq_h = Wq[:, sl]
        wk_h = Wk[:, sl]
        m = {
            "qT": qT, "kT": kT, "vT": vT,
            "wq": chunked(np.concatenate([wq_h, wq_h], 1)).astype(f16),
            "wk": chunked(np.concatenate([wk_h, wk_h], 1)).astype(f16),
            "wv": chunked(Wv[:, sl]).astype(f16),
            "bq": np.concatenate([bq[sl], bq[sl]]).reshape(128, 1).copy(),
            "bk": np.concatenate([bk[sl], bk[sl]]).reshape(128, 1).copy(),
            "bv": bv[sl].reshape(64, 1).copy(),
            "wg": np.concatenate([WG[h], WG[h]], 0).astype(f16),  # [128, 256]
            "bg": np.ascontiguousarray(
                bG[h].reshape(2, 128).T).astype(f32),     # [128, 2]
            "wfc": Wfc[h * 256:(h + 1) * 256, :].astype(f16),
        }
        maps.append(m)
    return maps


_compiled = None
last_results = None


def get_compiled():
    global _compiled
    if _compiled is None:
        _compiled = build_program()
    return _compiled


def kernel(**inputs):
    global last_results
    nc = get_compiled()
    in_maps = shard_inputs(inputs)
    last_results = bass_utils.run_bass_kernel_spmd(
        nc, in_maps, core_ids=list(range(8)))
    bfc = np.asarray(inputs["bfc"], np.float32)
    acc = np.zeros((N, E), np.float64)
    for res in last_results.results:
        acc += res["out"].astype(np.float64)
    full = (acc + bfc[None, :].astype(np.float64)).astype(np.float32)
    return full.reshape(1, N, E)
